# revision 11
# baseline (speedup 1.0000x reference)
"""CoxPH loss (with tie handling) on 8 Trainium2 NeuronCores — v4.

Math (identical to the validated v1 decomposition):

  Sort descending by time so the at-risk suffix sums become prefix sums.
    total = sum_i w_i*h_i - sum_j c_j*ln(Q_j)
  with w_i = e_i*n_g(i), c_j = n_g^2 at tie-group-start positions (0
  elsewhere), Q_j = prefix sum of exp(h) in time-descending order.
    loss = -total/n_events + 1e-4*sqrt(sum h^2)

Implementation strategy (driven by the TimelineSim V2 cost model: all DMA
transfers serialize on one shared DMA_ENGINES device at ~360 GB/s; compute
cost counts free-dim elements only; every DMA->compute edge pays a 900ns
semaphore):

  * fp8 h/w (e3m4: |h|<5.2, w<=7 exact), E=exp(h) e4m3 (max ~158 < 240),
    c/lnQ bf16.  Host-simulated pipeline rel err ~1.2e-4 (gate 2e-2).
  * Block-major layout: per core 8 blocks x [128 x 1024]; global time
    order = (core, block, partition, column).  Per-block partition
    offsets need only the block's own row sums (scan last column), so
    the Ln-bias machinery pipelines per block.  DMA granularity is a
    2-block pair; the host pre-swaps (block, partition) per pair so the
    SBUF pair tile and the DRAM rows flatten identically.
  * Launch 1: exp on ACT (fp8 out) -> E8 to DRAM; per-core S on the idle
    Pool engine (XYZWC reduce per pair); T1 = sum w*h and SSQ = sum h^2
    on the idle PE as accumulated [128x128] fp8 matmuls; raw PSUM
    matrices shipped out, host sums their diagonals.
  * host: 8 scalar adds -> per-core scan offsets (device collectives
    cost 15-28us in this cost model; the host hop is free).
  * Launch 2: per-block DVE scans (f32 accumulate); per-block bias =
    tri@qlast_b + sum_{b'<b} allones@qlast_b' + onesrow@offc, one PE
    psum chain per block (no cross-engine carry); Ln (bias) -> bf16;
    T2 = sum c*lnQ as bf16 PE matmul chain; raw PSUM out.

Runtime constraints (probed on this stack):
  * Pool/gpsimd cannot run tensor_tensor_scan (HW ISA check) — scans are
    DVE-only.  Activation bias must live in SBUF (PSUM rejected), so one
    psum->sbuf copy per block remains.
  * collective_compute fails at LoadExecutable under axon/PJRT; the
    cross-core scalar goes through the host between the two launches.
"""

import numpy as np

N = 8388608
CORES = 8
P = 128
C = 8192           # free-dim elements per partition per core
NBLK = 8
BS = C // NBLK     # 1024
NPAIR = NBLK // 2  # DMA granularity: 2 blocks per transfer
SUB = BS // P      # 8 matmul sub-chunks of 128 per block

_cache = {}


def _build_launch1():
    """Per core: h8,w8 [NPAIR*P, 2*BS] e3m4 in (pair layout); E8 out;
    part1 [P, 4 + 2P] f32 out = [S per pair (row 0) | T1 psum | SSQ psum]."""
    import concourse.bacc as bacc
    import concourse.tile as tile
    from concourse import mybir
    from contextlib import ExitStack

    f32 = mybir.dt.float32
    f8e3 = mybir.dt.float8e3
    f8e4 = mybir.dt.float8e4
    nc = bacc.Bacc("TRN2", debug=False, enable_asserts=False,
                   target_bir_lowering=False, num_devices=CORES)
    h_d = nc.dram_tensor("h", [NPAIR * P, 2 * BS], f8e3,
                         kind="ExternalInput").ap()
    w_d = nc.dram_tensor("w", [NPAIR * P, 2 * BS], f8e3,
                         kind="ExternalInput").ap()
    e_d = nc.dram_tensor("e8", [NPAIR * P, 2 * BS], f8e4,
                         kind="ExternalOutput").ap()
    p_d = nc.dram_tensor("part1", [P, NPAIR + 2 * P], f32,
                         kind="ExternalOutput").ap()

    with tile.TileContext(nc) as tc, ExitStack() as ctx:
        big = ctx.enter_context(tc.tile_pool(name="big", bufs=1))
        small = ctx.enter_context(tc.tile_pool(name="small", bufs=1))
        psum = ctx.enter_context(tc.tile_pool(name="psum", bufs=1, space="PSUM"))

        h_t = big.tile([P, C], f8e3)
        w_t = big.tile([P, C], f8e3)
        e_t = big.tile([P, C], f8e4)
        part = small.tile([P, NPAIR + 2 * P], f32)

        ps_t1 = psum.tile([P, P], f32)
        ps_sq = psum.tile([P, P], f32)

        for q in range(NPAIR):
            sl = slice(q * 2 * BS, (q + 1) * 2 * BS)
            rows = slice(q * P, (q + 1) * P)
            nc.sync.dma_start(h_t[:, sl], h_d[rows, :])
        for q in range(NPAIR):
            sl = slice(q * 2 * BS, (q + 1) * 2 * BS)
            rows = slice(q * P, (q + 1) * P)
            nc.sync.dma_start(w_t[:, sl], w_d[rows, :])

        nmm = NPAIR * 2 * SUB  # 64 per chain
        for q in range(NPAIR):
            sl = slice(q * 2 * BS, (q + 1) * 2 * BS)
            rows = slice(q * P, (q + 1) * P)
            nc.scalar.activation(e_t[:, sl], h_t[:, sl],
                                 mybir.ActivationFunctionType.Exp)
            nc.sync.dma_start(e_d[rows, :], e_t[:, sl])
            # per-pair total of exp on the otherwise idle Pool engine
            nc.gpsimd.tensor_reduce(part[0:1, q:q + 1], e_t[:, sl],
                                    mybir.AxisListType.XYZWC,
                                    mybir.AluOpType.add)
            for s in range(2 * SUB):
                ms = slice(q * 2 * BS + s * P, q * 2 * BS + (s + 1) * P)
                i = q * 2 * SUB + s
                # SSQ first: it only needs h, so PE starts before w lands
                nc.tensor.matmul(ps_sq[:], h_t[:, ms], h_t[:, ms],
                                 start=(i == 0), stop=(i == nmm - 1))
                nc.tensor.matmul(ps_t1[:], w_t[:, ms], h_t[:, ms],
                                 start=(i == 0), stop=(i == nmm - 1))

        nc.vector.tensor_scalar_add(part[:, NPAIR:NPAIR + P], ps_t1[:], 0.0)
        nc.vector.tensor_scalar_add(part[:, NPAIR + P:NPAIR + 2 * P],
                                    ps_sq[:], 0.0)
        nc.sync.dma_start(p_d, part[:])

    nc.compile()
    return nc


def _build_launch2():
    """Per core: E8, c16 [NPAIR*P, 2*BS] in (pair layout); smalls packed
    [P, 2P+1] = [tri | all-ones | offc broadcast]; part2 [P, P] f32 out
    (raw T2 psum; host sums the diagonal)."""
    import concourse.bacc as bacc
    import concourse.tile as tile
    from concourse import mybir
    from contextlib import ExitStack

    f32 = mybir.dt.float32
    bf16 = mybir.dt.bfloat16
    f8e4 = mybir.dt.float8e4
    nc = bacc.Bacc("TRN2", debug=False, enable_asserts=False,
                   target_bir_lowering=False, num_devices=CORES)
    e_d = nc.dram_tensor("e8", [NPAIR * P, 2 * BS], f8e4,
                         kind="ExternalInput").ap()
    c_d = nc.dram_tensor("c16", [NPAIR * P, 2 * BS], bf16,
                         kind="ExternalInput").ap()
    sm_d = nc.dram_tensor("smalls", [P, 2 * P + 1], f32,
                          kind="ExternalInput").ap()
    p_d = nc.dram_tensor("part2", [P, P], f32, kind="ExternalOutput").ap()

    with tile.TileContext(nc) as tc, ExitStack() as ctx:
        big = ctx.enter_context(tc.tile_pool(name="big", bufs=1))
        small = ctx.enter_context(tc.tile_pool(name="small", bufs=1))
        psum = ctx.enter_context(tc.tile_pool(name="psum", bufs=1, space="PSUM"))
        psum2 = ctx.enter_context(tc.tile_pool(name="psum2", bufs=2,
                                               space="PSUM"))

        e_t = big.tile([P, C], f8e4)
        q_t = big.tile([P, C], f32)
        c_t = big.tile([P, C], bf16)
        l_t = big.tile([P, C], bf16)
        sm_t = small.tile([P, 2 * P + 1], f32)
        tri_t = sm_t[:, 0:P]              # [k,i]=1 iff k<i
        om_t = sm_t[:, P:2 * P]           # all-ones [P,P]
        orow_t = sm_t[0:1, P:2 * P]       # its row 0 = ones row [1,P]
        off_t = sm_t[0:1, 2 * P:2 * P + 1]  # offc at [0, 2P]

        # E pairs first (scans gate everything), then constants, then c
        # (only consumed by the trailing T2 matmuls) — one queue, strict
        # order on the shared DMA device.
        for q in range(NPAIR):
            sl = slice(q * 2 * BS, (q + 1) * 2 * BS)
            rows = slice(q * P, (q + 1) * P)
            nc.sync.dma_start(e_t[:, sl], e_d[rows, :])
        nc.sync.dma_start(sm_t[:], sm_d)
        for q in range(NPAIR):
            sl = slice(q * 2 * BS, (q + 1) * 2 * BS)
            rows = slice(q * P, (q + 1) * P)
            nc.sync.dma_start(c_t[:, sl], c_d[rows, :])

        qlasts = []
        ps_t2 = psum.tile([P, P], f32)
        nmm = NBLK * SUB
        for b in range(NBLK):
            sl = slice(b * BS, (b + 1) * BS)
            nc.vector.tensor_tensor_scan(
                q_t[:, sl], e_t[:, sl], e_t[:, sl], 0.0,
                mybir.AluOpType.add, mybir.AluOpType.bypass)
            qlast = q_t[:, (b + 1) * BS - 1:(b + 1) * BS]
            qlasts.append(qlast)
            # per-block bias entirely on PE: partition offsets within the
            # block + totals of earlier blocks + the per-core offset
            with tc.high_priority():
                pacc = psum2.tile([P, 1], f32, tag="pacc")
                nc.tensor.matmul(pacc[:], tri_t, qlast, start=True,
                                 stop=False)
                for b2 in range(b):
                    nc.tensor.matmul(pacc[:], om_t, qlasts[b2], start=False,
                                     stop=False)
                nc.tensor.matmul(pacc[:], orow_t, off_t, start=False,
                                 stop=True)
                off_sb = small.tile([P, 1], f32, tag=f"offsb{b}")
                nc.vector.tensor_scalar_add(off_sb[:], pacc[:], 0.0)
            nc.scalar.activation(l_t[:, sl], q_t[:, sl],
                                 mybir.ActivationFunctionType.Ln,
                                 bias=off_sb[:], scale=1.0)
            for s in range(SUB):
                ms = slice(b * BS + s * P, b * BS + (s + 1) * P)
                i = b * SUB + s
                nc.tensor.matmul(ps_t2[:], c_t[:, ms], l_t[:, ms],
                                 start=(i == 0), stop=(i == nmm - 1))

        part = small.tile([P, P], f32)
        nc.vector.tensor_scalar_add(part[:], ps_t2[:], 0.0)
        nc.sync.dma_start(p_d, part[:])

    nc.compile()
    return nc


def _get_programs():
    if "progs" not in _cache:
        _cache["progs"] = (_build_launch1(), _build_launch2())
    return _cache["progs"]


LAST = {}


def _pair_layout(a):
    """[CORES, NBLK*P, BS] block-major -> [CORES, NPAIR*P, 2*BS] where
    row q*P+p holds blocks 2q,2q+1 of partition p side by side (matches
    the SBUF pair-tile flattening order)."""
    return np.ascontiguousarray(
        a.reshape(CORES, NPAIR, 2, P, BS)
         .transpose(0, 1, 3, 2, 4)
         .reshape(CORES, NPAIR * P, 2 * BS))


def kernel(hazard_pred, times, events):
    import ml_dtypes
    from concourse.bass_utils import run_bass_kernel_spmd

    np_e3 = ml_dtypes.float8_e3m4
    np_bf = ml_dtypes.bfloat16

    h = np.asarray(hazard_pred, dtype=np.float32)
    t = np.asarray(times, dtype=np.float32)
    e = np.asarray(events, dtype=np.int32)
    assert h.shape == (N,)

    # ---- host bookkeeping: ordering + tie structure (integer only) ----
    order = np.argsort(t, kind="stable")
    t_s = t[order]
    h_s = h[order]
    e_s = e[order]
    first = np.searchsorted(t_s, t_s, side="left")     # group-start index
    n_at_start = np.bincount(first, weights=e_s.astype(np.float64),
                             minlength=N)              # events per group
    m = n_at_start[first]                              # broadcast to members
    w = (e_s * m).astype(np.float32)                   # e_i * n_g(i)
    cvec = np.zeros(N, dtype=np.float32)
    starts = first == np.arange(N)
    cvec[starts] = (n_at_start[starts] ** 2).astype(np.float32)
    n_events = int(e.sum())

    # time-DESCENDING block-major layout: (core, block, partition, column)
    hd = h_s[::-1].reshape(CORES, NBLK * P, BS)
    wd = w[::-1].reshape(CORES, NBLK * P, BS)
    cd = cvec[::-1].reshape(CORES, NBLK * P, BS)
    h8 = _pair_layout(hd.astype(np.float32)).astype(np_e3)
    w8 = _pair_layout(wd.astype(np.float32)).astype(np_e3)
    c16 = _pair_layout(cd.astype(np.float32)).astype(np_bf)

    tri = np.triu(np.ones((P, P), dtype=np.float32), 1)  # [k,i]=1 iff k<i

    nc1, nc2 = _get_programs()
    core_ids = list(range(CORES))

    in1 = [{"h": np.ascontiguousarray(h8[i]),
            "w": np.ascontiguousarray(w8[i])}
           for i in range(CORES)]
    r1 = run_bass_kernel_spmd(nc1, in1, core_ids=core_ids)
    part1 = np.stack([r1.results[i]["part1"] for i in range(CORES)])
    E8 = [r1.results[i]["e8"] for i in range(CORES)]

    S = part1[:, 0, 0:NPAIR].sum(axis=1, dtype=np.float64)   # per-core
    M1 = part1[:, :, NPAIR:NPAIR + P]
    M2 = part1[:, :, NPAIR + P:NPAIR + 2 * P]
    idx = np.arange(P)
    T1 = M1[:, idx, idx].sum(dtype=np.float64)
    SSQ = M2[:, idx, idx].sum(dtype=np.float64)

    # descending-order prefix offsets across cores (8 scalar adds)
    offs = np.concatenate([[0.0], np.cumsum(S)[:-1]]).astype(np.float32)

    def smalls(off):
        sm = np.ones((P, 2 * P + 1), dtype=np.float32)
        sm[:, 0:P] = tri
        sm[:, 2 * P] = off
        return sm

    in2 = [{"e8": np.ascontiguousarray(E8[i]),
            "c16": np.ascontiguousarray(c16[i]),
            "smalls": smalls(offs[i])}
           for i in range(CORES)]
    r2 = run_bass_kernel_spmd(nc2, in2, core_ids=core_ids)
    part2 = np.stack([r2.results[i]["part2"] for i in range(CORES)])
    T2 = part2[:, idx, idx].sum(dtype=np.float64)

    LAST.clear()
    LAST.update({"r1": r1, "r2": r2})

    total = T1 - T2
    loss = -total / n_events + 1e-4 * np.sqrt(SSQ)
    return np.float32(loss)


# revision 15
# speedup vs baseline: 1.0963x; 1.0963x over previous
"""CoxPH loss (with tie handling) on 8 Trainium2 NeuronCores — v4.

Math (identical to the validated v1 decomposition):

  Sort descending by time so the at-risk suffix sums become prefix sums.
    total = sum_i w_i*h_i - sum_j c_j*ln(Q_j)
  with w_i = e_i*n_g(i), c_j = n_g^2 at tie-group-start positions (0
  elsewhere), Q_j = prefix sum of exp(h) in time-descending order.
    loss = -total/n_events + 1e-4*sqrt(sum h^2)

Implementation strategy (driven by the TimelineSim V2 cost model: all DMA
transfers serialize on one shared DMA_ENGINES device at ~360 GB/s; compute
cost counts free-dim elements only; every DMA->compute edge pays a 900ns
semaphore):

  * fp8 h/w (e3m4: |h|<5.2, w<=7 exact), E=exp(h) e4m3 (max ~158 < 240),
    c/lnQ bf16.  Host-simulated pipeline rel err ~1.2e-4 (gate 2e-2).
  * Block-major layout: per core 8 blocks x [128 x 1024]; global time
    order = (core, block, partition, column).  Per-block partition
    offsets need only the block's own row sums (scan last column), so
    the Ln-bias machinery pipelines per block.  DMA granularity is a
    2-block pair; the host pre-swaps (block, partition) per pair so the
    SBUF pair tile and the DRAM rows flatten identically.
  * Launch 1: exp on ACT (fp8 out) -> E8 to DRAM; per-core S on the idle
    Pool engine (XYZWC reduce per pair); T1 = sum w*h and SSQ = sum h^2
    on the idle PE as accumulated [128x128] fp8 matmuls; raw PSUM
    matrices shipped out, host sums their diagonals.
  * host: 8 scalar adds -> per-core scan offsets (device collectives
    cost 15-28us in this cost model; the host hop is free).
  * Launch 2: per-block DVE scans (f32 accumulate); per-block bias =
    tri@qlast_b + sum_{b'<b} allones@qlast_b' + onesrow@offc, one PE
    psum chain per block (no cross-engine carry); Ln (bias) -> bf16;
    T2 = sum c*lnQ as bf16 PE matmul chain; raw PSUM out.

Runtime constraints (probed on this stack):
  * Pool/gpsimd cannot run tensor_tensor_scan (HW ISA check) — scans are
    DVE-only.  Activation bias must live in SBUF (PSUM rejected), so one
    psum->sbuf copy per block remains.
  * collective_compute fails at LoadExecutable under axon/PJRT; the
    cross-core scalar goes through the host between the two launches.
"""

import numpy as np

N = 8388608
CORES = 8
P = 128
C = 8192           # free-dim elements per partition per core
NBLK = 8
BS = C // NBLK     # 1024
NPAIR = NBLK // 2  # DMA granularity: 2 blocks per transfer
SUB = BS // P      # 8 matmul sub-chunks of 128 per block

_cache = {}


def _build_launch1():
    """Per core: h8,w8 [NPAIR*P, 2*BS] e3m4 in (pair layout); E8 out;
    part1 [P, 4 + 2P] f32 out = [S per pair (row 0) | T1 psum | SSQ psum]."""
    import concourse.bacc as bacc
    import concourse.tile as tile
    from concourse import mybir
    from contextlib import ExitStack

    f32 = mybir.dt.float32
    f8e3 = mybir.dt.float8e3
    f8e4 = mybir.dt.float8e4
    nc = bacc.Bacc("TRN2", debug=False, enable_asserts=False,
                   target_bir_lowering=False, num_devices=CORES)
    h_d = nc.dram_tensor("h", [NPAIR * P, 2 * BS], f8e3,
                         kind="ExternalInput").ap()
    w_d = nc.dram_tensor("w", [NPAIR * P, 2 * BS], f8e3,
                         kind="ExternalInput").ap()
    e_d = nc.dram_tensor("e8", [NPAIR * P, 2 * BS], f8e4,
                         kind="ExternalOutput").ap()
    p_d = nc.dram_tensor("part1", [P, NPAIR + 2 * P], f32,
                         kind="ExternalOutput").ap()

    with tile.TileContext(nc) as tc, ExitStack() as ctx:
        big = ctx.enter_context(tc.tile_pool(name="big", bufs=1))
        small = ctx.enter_context(tc.tile_pool(name="small", bufs=1))
        psum = ctx.enter_context(tc.tile_pool(name="psum", bufs=1, space="PSUM"))

        h_t = big.tile([P, C], f8e3)
        w_t = big.tile([P, C], f8e3)
        e_t = big.tile([P, C], f8e4)
        part = small.tile([P, NPAIR + 2 * P], f32)
        esum = part[:, 0:NPAIR]

        ps_t1 = psum.tile([P, P], f32)
        ps_sq = psum.tile([P, P], f32)

        for q in range(NPAIR):
            sl = slice(q * 2 * BS, (q + 1) * 2 * BS)
            rows = slice(q * P, (q + 1) * P)
            nc.sync.dma_start(h_t[:, sl], h_d[rows, :])
        for q in range(NPAIR):
            sl = slice(q * 2 * BS, (q + 1) * 2 * BS)
            rows = slice(q * P, (q + 1) * P)
            nc.sync.dma_start(w_t[:, sl], w_d[rows, :])

        nmm = NPAIR * 2 * SUB  # 64 per chain
        for q in range(NPAIR):
            sl = slice(q * 2 * BS, (q + 1) * 2 * BS)
            rows = slice(q * P, (q + 1) * P)
            nc.scalar.activation(e_t[:, sl], h_t[:, sl],
                                 mybir.ActivationFunctionType.Exp,
                                 accum_out=esum[:, q:q + 1])
            nc.sync.dma_start(e_d[rows, :], e_t[:, sl])
            for s in range(2 * SUB):
                ms = slice(q * 2 * BS + s * P, q * 2 * BS + (s + 1) * P)
                i = q * 2 * SUB + s
                # SSQ first: it only needs h, so PE starts before w lands
                nc.tensor.matmul(ps_sq[:], h_t[:, ms], h_t[:, ms],
                                 start=(i == 0), stop=(i == nmm - 1))
                nc.tensor.matmul(ps_t1[:], w_t[:, ms], h_t[:, ms],
                                 start=(i == 0), stop=(i == nmm - 1))

        nc.vector.tensor_scalar_add(part[:, NPAIR:NPAIR + P], ps_t1[:], 0.0)
        nc.vector.tensor_scalar_add(part[:, NPAIR + P:NPAIR + 2 * P],
                                    ps_sq[:], 0.0)
        nc.sync.dma_start(p_d, part[:])

    nc.compile()
    return nc


def _build_launch2():
    """Per core: E8, c16 [NPAIR*P, 2*BS] in (pair layout); smalls packed
    [P, 2P+1] = [tri | all-ones | offc broadcast]; part2 [P, P] f32 out
    (raw T2 psum; host sums the diagonal)."""
    import concourse.bacc as bacc
    import concourse.tile as tile
    from concourse import mybir
    from contextlib import ExitStack

    f32 = mybir.dt.float32
    bf16 = mybir.dt.bfloat16
    f8e4 = mybir.dt.float8e4
    nc = bacc.Bacc("TRN2", debug=False, enable_asserts=False,
                   target_bir_lowering=False, num_devices=CORES)
    e_d = nc.dram_tensor("e8", [NPAIR * P, 2 * BS], f8e4,
                         kind="ExternalInput").ap()
    c_d = nc.dram_tensor("c16", [NPAIR * P, 2 * BS], bf16,
                         kind="ExternalInput").ap()
    sm_d = nc.dram_tensor("smalls", [P, 2 * P + 1], f32,
                          kind="ExternalInput").ap()
    p_d = nc.dram_tensor("part2", [P, P], f32, kind="ExternalOutput").ap()

    with tile.TileContext(nc) as tc, ExitStack() as ctx:
        big = ctx.enter_context(tc.tile_pool(name="big", bufs=1))
        small = ctx.enter_context(tc.tile_pool(name="small", bufs=1))
        psum = ctx.enter_context(tc.tile_pool(name="psum", bufs=1, space="PSUM"))
        psum2 = ctx.enter_context(tc.tile_pool(name="psum2", bufs=2,
                                               space="PSUM"))

        e_t = big.tile([P, C], f8e4)
        q_t = big.tile([P, C], f32)
        c_t = big.tile([P, C], bf16)
        l_t = big.tile([P, C], bf16)
        sm_t = small.tile([P, 2 * P + 1], f32)
        tri_t = sm_t[:, 0:P]              # [k,i]=1 iff k<i
        om_t = sm_t[:, P:2 * P]           # all-ones [P,P]
        orow_t = sm_t[0:1, P:2 * P]       # its row 0 = ones row [1,P]
        off_t = sm_t[0:1, 2 * P:2 * P + 1]  # offc at [0, 2P]

        # E pairs first on SP (scans gate everything); the constants ride
        # the ACT HWDGE queue so they land right after E0 without pushing
        # the E pairs back; c pairs last (only the trailing T2 needs them).
        nc.scalar.dma_start(sm_t[:], sm_d)
        for q in range(NPAIR):
            sl = slice(q * 2 * BS, (q + 1) * 2 * BS)
            rows = slice(q * P, (q + 1) * P)
            nc.sync.dma_start(e_t[:, sl], e_d[rows, :])
        for q in range(NPAIR):
            sl = slice(q * 2 * BS, (q + 1) * 2 * BS)
            rows = slice(q * P, (q + 1) * P)
            nc.sync.dma_start(c_t[:, sl], c_d[rows, :])

        qlasts = []
        ps_t2 = psum.tile([P, P], f32)
        nmm = NBLK * SUB
        for b in range(NBLK):
            sl = slice(b * BS, (b + 1) * BS)
            nc.vector.tensor_tensor_scan(
                q_t[:, sl], e_t[:, sl], e_t[:, sl], 0.0,
                mybir.AluOpType.add, mybir.AluOpType.bypass)
            qlast = q_t[:, (b + 1) * BS - 1:(b + 1) * BS]
            qlasts.append(qlast)
            # per-block bias entirely on PE: partition offsets within the
            # block + totals of earlier blocks + the per-core offset
            with tc.high_priority():
                pacc = psum2.tile([P, 1], f32, tag="pacc")
                nc.tensor.matmul(pacc[:], tri_t, qlast, start=True,
                                 stop=False)
                for b2 in range(b):
                    nc.tensor.matmul(pacc[:], om_t, qlasts[b2], start=False,
                                     stop=False)
                nc.tensor.matmul(pacc[:], orow_t, off_t, start=False,
                                 stop=True)
                off_sb = small.tile([P, 1], f32, tag=f"offsb{b}")
                nc.vector.tensor_scalar_add(off_sb[:], pacc[:], 0.0)
            nc.scalar.activation(l_t[:, sl], q_t[:, sl],
                                 mybir.ActivationFunctionType.Ln,
                                 bias=off_sb[:], scale=1.0)
            for s in range(SUB):
                ms = slice(b * BS + s * P, b * BS + (s + 1) * P)
                i = b * SUB + s
                nc.tensor.matmul(ps_t2[:], c_t[:, ms], l_t[:, ms],
                                 start=(i == 0), stop=(i == nmm - 1))

        part = small.tile([P, P], f32)
        nc.vector.tensor_scalar_add(part[:], ps_t2[:], 0.0)
        nc.sync.dma_start(p_d, part[:])

    nc.compile()
    return nc


def _get_programs():
    if "progs" not in _cache:
        _cache["progs"] = (_build_launch1(), _build_launch2())
    return _cache["progs"]


LAST = {}


def _pair_layout(a):
    """[CORES, NBLK*P, BS] block-major -> [CORES, NPAIR*P, 2*BS] where
    row q*P+p holds blocks 2q,2q+1 of partition p side by side (matches
    the SBUF pair-tile flattening order)."""
    return np.ascontiguousarray(
        a.reshape(CORES, NPAIR, 2, P, BS)
         .transpose(0, 1, 3, 2, 4)
         .reshape(CORES, NPAIR * P, 2 * BS))


def kernel(hazard_pred, times, events):
    import ml_dtypes
    from concourse.bass_utils import run_bass_kernel_spmd

    np_e3 = ml_dtypes.float8_e3m4
    np_bf = ml_dtypes.bfloat16

    h = np.asarray(hazard_pred, dtype=np.float32)
    t = np.asarray(times, dtype=np.float32)
    e = np.asarray(events, dtype=np.int32)
    assert h.shape == (N,)

    # ---- host bookkeeping: ordering + tie structure (integer only) ----
    order = np.argsort(t, kind="stable")
    t_s = t[order]
    h_s = h[order]
    e_s = e[order]
    first = np.searchsorted(t_s, t_s, side="left")     # group-start index
    n_at_start = np.bincount(first, weights=e_s.astype(np.float64),
                             minlength=N)              # events per group
    m = n_at_start[first]                              # broadcast to members
    w = (e_s * m).astype(np.float32)                   # e_i * n_g(i)
    cvec = np.zeros(N, dtype=np.float32)
    starts = first == np.arange(N)
    cvec[starts] = (n_at_start[starts] ** 2).astype(np.float32)
    n_events = int(e.sum())

    # time-DESCENDING block-major layout: (core, block, partition, column)
    hd = h_s[::-1].reshape(CORES, NBLK * P, BS)
    wd = w[::-1].reshape(CORES, NBLK * P, BS)
    cd = cvec[::-1].reshape(CORES, NBLK * P, BS)
    h8 = _pair_layout(hd.astype(np.float32)).astype(np_e3)
    w8 = _pair_layout(wd.astype(np.float32)).astype(np_e3)
    c16 = _pair_layout(cd.astype(np.float32)).astype(np_bf)

    tri = np.triu(np.ones((P, P), dtype=np.float32), 1)  # [k,i]=1 iff k<i

    nc1, nc2 = _get_programs()
    core_ids = list(range(CORES))

    in1 = [{"h": np.ascontiguousarray(h8[i]),
            "w": np.ascontiguousarray(w8[i])}
           for i in range(CORES)]
    r1 = run_bass_kernel_spmd(nc1, in1, core_ids=core_ids)
    part1 = np.stack([r1.results[i]["part1"] for i in range(CORES)])
    E8 = [r1.results[i]["e8"] for i in range(CORES)]

    S = part1[:, :, 0:NPAIR].sum(axis=(1, 2), dtype=np.float64)  # per-core
    M1 = part1[:, :, NPAIR:NPAIR + P]
    M2 = part1[:, :, NPAIR + P:NPAIR + 2 * P]
    idx = np.arange(P)
    T1 = M1[:, idx, idx].sum(dtype=np.float64)
    SSQ = M2[:, idx, idx].sum(dtype=np.float64)

    # descending-order prefix offsets across cores (8 scalar adds)
    offs = np.concatenate([[0.0], np.cumsum(S)[:-1]]).astype(np.float32)

    def smalls(off):
        sm = np.ones((P, 2 * P + 1), dtype=np.float32)
        sm[:, 0:P] = tri
        sm[:, 2 * P] = off
        return sm

    in2 = [{"e8": np.ascontiguousarray(E8[i]),
            "c16": np.ascontiguousarray(c16[i]),
            "smalls": smalls(offs[i])}
           for i in range(CORES)]
    r2 = run_bass_kernel_spmd(nc2, in2, core_ids=core_ids)
    part2 = np.stack([r2.results[i]["part2"] for i in range(CORES)])
    T2 = part2[:, idx, idx].sum(dtype=np.float64)

    LAST.clear()
    LAST.update({"r1": r1, "r2": r2})

    total = T1 - T2
    loss = -total / n_events + 1e-4 * np.sqrt(SSQ)
    return np.float32(loss)


# revision 16
# speedup vs baseline: 1.1118x; 1.0142x over previous
"""CoxPH loss (with tie handling) on 8 Trainium2 NeuronCores — v6.

Math (identical to the validated v1 decomposition):

  Sort descending by time so the at-risk suffix sums become prefix sums.
    total = sum_i w_i*h_i - sum_j c_j*ln(Q_j)
  with w_i = e_i*n_g(i), c_j = n_g^2 at tie-group-start positions (0
  elsewhere), Q_j = prefix sum of exp(h) in time-descending order.
    loss = -total/n_events + 1e-4*sqrt(sum h^2)

Implementation strategy (driven by the TimelineSim V2 cost model: all DMA
transfers serialize on one shared DMA_ENGINES device at ~360 GB/s; compute
cost counts free-dim elements only; every DMA->compute edge pays a 900ns
semaphore; PE runs at 1/4 speed until it has been busy ~3us):

  * fp8 h/w (e3m4: |h|<5.2, w<=7 exact), E=exp(h) e4m3 (max ~158 < 240),
    c/lnQ bf16.  Host-simulated pipeline rel err ~1.2e-4 (gate 2e-2).
  * Per-core layout is the SBUF image itself [128 x 8192]: global time
    order = (core, block, partition, column-within-block), with variable
    block widths (small first/last blocks so the Ln chain starts early
    and finishes with a short trailing block).
  * Launch 1: exp on ACT (fp8 out, accum_out -> per-pair sums) -> E8 to
    DRAM; T1 = sum w*h and SSQ = sum h^2 on the idle PE as accumulated
    [128x128] fp8 matmuls, preceded by a chain of zero-matmuls that only
    warms the PE p-state (they add exact zeros into the T1 accumulator);
    raw PSUM matrices shipped out, host sums their diagonals.
  * host: 8 scalar adds -> per-core scan offsets (device collectives
    cost 15-28us in this cost model; the host hop is free).
  * Launch 2: per-block DVE scans (f32 accumulate); per-block bias =
    tri@qlast_b + sum_{b'<b} allones@qlast_b' + onesrow@offc as one PE
    psum chain per block; Ln (bias) -> bf16; T2 = sum c*lnQ as bf16 PE
    matmul chains into two PSUM accumulators (early copy-out of the
    first); raw PSUM out, host sums diagonals.

Runtime constraints (probed on this stack):
  * Pool/gpsimd cannot run tensor_tensor_scan or free-axis reduces
    (HW ISA check / cost blowup) — scans are DVE-only.
  * Activation bias must live in SBUF (PSUM rejected) — one psum->sbuf
    copy per block remains, scheduled by Tile wherever it fits.
  * collective_compute fails at LoadExecutable under axon/PJRT; the
    cross-core scalar goes through the host between the two launches.
"""

import numpy as np

N = 8388608
CORES = 8
P = 128
C = 8192            # free-dim elements per partition per core
NPAIR = 4           # DMA chunk count per big tensor (2048 cols each)
DCH = C // NPAIR    # 2048
# Scan/Ln block widths (sum = C).  Small first blocks let the Ln chain
# start early; small last blocks shorten the post-scan trailing chain.
BLOCKS = [512, 512, 1024, 1024, 1024, 1024, 1024, 1024, 512, 512]
assert sum(BLOCKS) == C
NWARM = 12          # PE p-state warm-up matmuls in launch 1

_cache = {}


def _build_launch1():
    """Per core: h8,w8 [P, C] e3m4 in; E8 [P, C] e4m3 out; part1
    [P, NPAIR + 2P] f32 out = [esum per DMA chunk | T1 psum | SSQ psum]."""
    import concourse.bacc as bacc
    import concourse.tile as tile
    from concourse import mybir
    from contextlib import ExitStack

    f32 = mybir.dt.float32
    f8e3 = mybir.dt.float8e3
    f8e4 = mybir.dt.float8e4
    nc = bacc.Bacc("TRN2", debug=False, enable_asserts=False,
                   target_bir_lowering=False, num_devices=CORES)
    h_d = nc.dram_tensor("h", [P, C], f8e3, kind="ExternalInput").ap()
    w_d = nc.dram_tensor("w", [P, C], f8e3, kind="ExternalInput").ap()
    e_d = nc.dram_tensor("e8", [P, C], f8e4, kind="ExternalOutput").ap()
    p_d = nc.dram_tensor("part1", [P, NPAIR + 2 * P], f32,
                         kind="ExternalOutput").ap()

    with tile.TileContext(nc) as tc, ExitStack() as ctx:
        big = ctx.enter_context(tc.tile_pool(name="big", bufs=1))
        small = ctx.enter_context(tc.tile_pool(name="small", bufs=1))
        psum = ctx.enter_context(tc.tile_pool(name="psum", bufs=1, space="PSUM"))

        h_t = big.tile([P, C], f8e3)
        w_t = big.tile([P, C], f8e3)
        e_t = big.tile([P, C], f8e4)
        part = small.tile([P, NPAIR + 2 * P], f32)
        esum = part[:, 0:NPAIR]
        zero_t = small.tile([P, P], f32)

        ps_t1 = psum.tile([P, P], f32)
        ps_sq = psum.tile([P, P], f32)

        for q in range(NPAIR):
            sl = slice(q * DCH, (q + 1) * DCH)
            nc.sync.dma_start(h_t[:, sl], h_d[:, sl])
        for q in range(NPAIR):
            sl = slice(q * DCH, (q + 1) * DCH)
            nc.sync.dma_start(w_t[:, sl], w_d[:, sl])

        # PE p-state warm-up: f32 zero-matmuls accumulate exact zeros into
        # the T1 psum while the first DMAs are in flight, so the real fp8
        # chains below run at full clock.
        nc.vector.memset(zero_t[:], 0.0)
        for i in range(NWARM):
            nc.tensor.matmul(ps_t1[:], zero_t[:], zero_t[:],
                             start=(i == 0), stop=False)

        nmm = NPAIR * (DCH // P)  # 64 per chain
        for q in range(NPAIR):
            sl = slice(q * DCH, (q + 1) * DCH)
            nc.scalar.activation(e_t[:, sl], h_t[:, sl],
                                 mybir.ActivationFunctionType.Exp,
                                 accum_out=esum[:, q:q + 1])
            nc.sync.dma_start(e_d[:, sl], e_t[:, sl])
            for s in range(DCH // P):
                ms = slice(q * DCH + s * P, q * DCH + (s + 1) * P)
                i = q * (DCH // P) + s
                nc.tensor.matmul(ps_sq[:], h_t[:, ms], h_t[:, ms],
                                 start=(i == 0), stop=(i == nmm - 1))
                nc.tensor.matmul(ps_t1[:], w_t[:, ms], h_t[:, ms],
                                 start=False, stop=(i == nmm - 1))

        nc.vector.tensor_scalar_add(part[:, NPAIR:NPAIR + P], ps_t1[:], 0.0)
        nc.vector.tensor_scalar_add(part[:, NPAIR + P:NPAIR + 2 * P],
                                    ps_sq[:], 0.0)
        nc.sync.dma_start(p_d, part[:])

    nc.compile()
    return nc


def _build_launch2():
    """Per core: E8, c16 [P, C] in; smalls packed [P, 2P+1] = [tri |
    all-ones | offc broadcast]; part2 [P, 2P] f32 out (two raw T2 psum
    accumulators; host sums both diagonals)."""
    import concourse.bacc as bacc
    import concourse.tile as tile
    from concourse import mybir
    from contextlib import ExitStack

    f32 = mybir.dt.float32
    bf16 = mybir.dt.bfloat16
    f8e4 = mybir.dt.float8e4
    nc = bacc.Bacc("TRN2", debug=False, enable_asserts=False,
                   target_bir_lowering=False, num_devices=CORES)
    e_d = nc.dram_tensor("e8", [P, C], f8e4, kind="ExternalInput").ap()
    c_d = nc.dram_tensor("c16", [P, C], bf16, kind="ExternalInput").ap()
    sm_d = nc.dram_tensor("smalls", [P, 2 * P + 1], f32,
                          kind="ExternalInput").ap()
    p_d = nc.dram_tensor("part2", [P, 2 * P], f32, kind="ExternalOutput").ap()

    nblk = len(BLOCKS)
    starts = np.concatenate([[0], np.cumsum(BLOCKS)]).astype(int)

    with tile.TileContext(nc) as tc, ExitStack() as ctx:
        big = ctx.enter_context(tc.tile_pool(name="big", bufs=1))
        small = ctx.enter_context(tc.tile_pool(name="small", bufs=1))
        psum = ctx.enter_context(tc.tile_pool(name="psum", bufs=1, space="PSUM"))
        psum2 = ctx.enter_context(tc.tile_pool(name="psum2", bufs=2,
                                               space="PSUM"))

        e_t = big.tile([P, C], f8e4)
        q_t = big.tile([P, C], f32)
        c_t = big.tile([P, C], bf16)
        l_t = big.tile([P, C], bf16)
        sm_t = small.tile([P, 2 * P + 1], f32)
        tri_t = sm_t[:, 0:P]                 # [k,i]=1 iff k<i
        om_t = sm_t[:, P:2 * P]              # all-ones [P,P]
        orow_t = sm_t[0:1, P:2 * P]          # its row 0 = ones row [1,P]
        off_t = sm_t[0:1, 2 * P:2 * P + 1]   # offc at [0, 2P]

        # E chunks first on SP (scans gate everything); the constants ride
        # the ACT HWDGE queue so they land right after E0 without pushing
        # the E chunks back; c chunks last (only the trailing T2 needs them).
        nc.scalar.dma_start(sm_t[:], sm_d)
        for q in range(NPAIR):
            sl = slice(q * DCH, (q + 1) * DCH)
            nc.sync.dma_start(e_t[:, sl], e_d[:, sl])
        for q in range(NPAIR):
            sl = slice(q * DCH, (q + 1) * DCH)
            nc.sync.dma_start(c_t[:, sl], c_d[:, sl])

        qlasts = []
        ps_a = psum.tile([P, P], f32)
        ps_b = psum.tile([P, P], f32)
        nmm_a = sum(bs // P for bs in BLOCKS[:-1])
        nmm_b = BLOCKS[-1] // P
        part = small.tile([P, 2 * P], f32)
        ia = 0
        ib = 0
        for b in range(nblk):
            sl = slice(int(starts[b]), int(starts[b + 1]))
            nc.vector.tensor_tensor_scan(
                q_t[:, sl], e_t[:, sl], e_t[:, sl], 0.0,
                mybir.AluOpType.add, mybir.AluOpType.bypass)
            qlast = q_t[:, int(starts[b + 1]) - 1:int(starts[b + 1])]
            qlasts.append(qlast)
            # per-block bias entirely on PE: partition offsets within the
            # block + totals of earlier blocks + the per-core offset
            pacc = psum2.tile([P, 1], f32, tag="pacc")
            nc.tensor.matmul(pacc[:], tri_t, qlast, start=True, stop=False)
            for b2 in range(b):
                nc.tensor.matmul(pacc[:], om_t, qlasts[b2], start=False,
                                 stop=False)
            nc.tensor.matmul(pacc[:], orow_t, off_t, start=False, stop=True)
            off_sb = small.tile([P, 1], f32, tag=f"offsb{b}")
            nc.vector.tensor_scalar_add(off_sb[:], pacc[:], 0.0)
            nc.scalar.activation(l_t[:, sl], q_t[:, sl],
                                 mybir.ActivationFunctionType.Ln,
                                 bias=off_sb[:], scale=1.0)
            ps, last = (ps_b, nmm_b) if b == nblk - 1 else (ps_a, nmm_a)
            for s in range(BLOCKS[b] // P):
                ms = slice(int(starts[b]) + s * P, int(starts[b]) + (s + 1) * P)
                if b == nblk - 1:
                    nc.tensor.matmul(ps[:], c_t[:, ms], l_t[:, ms],
                                     start=(ib == 0), stop=(ib == last - 1))
                    ib += 1
                else:
                    nc.tensor.matmul(ps[:], c_t[:, ms], l_t[:, ms],
                                     start=(ia == 0), stop=(ia == last - 1))
                    ia += 1
            if b == nblk - 2:
                # first accumulator complete: copy it out while the last
                # block's Ln/T2 still run
                nc.vector.tensor_scalar_add(part[:, 0:P], ps_a[:], 0.0)

        nc.vector.tensor_scalar_add(part[:, P:2 * P], ps_b[:], 0.0)
        nc.sync.dma_start(p_d, part[:])

    nc.compile()
    return nc


def _get_programs():
    if "progs" not in _cache:
        _cache["progs"] = (_build_launch1(), _build_launch2())
    return _cache["progs"]


LAST = {}


def _image_layout(a):
    """[CORES, NBLK, P, bs] per-block arrays already concatenated as the
    SBUF image: input here is [CORES, N//CORES] flat in global descending
    order; emit [CORES, P, C] where columns are block-concatenated."""
    out = np.empty((CORES, P, C), dtype=a.dtype)
    pos = 0
    for b, bs in enumerate(BLOCKS):
        blk = a[:, pos * P:(pos + bs) * P]  # [CORES, P*bs] flat (p, x)
        out[:, :, pos:pos + bs] = blk.reshape(CORES, P, bs)
        pos += bs
    return out


def kernel(hazard_pred, times, events):
    import ml_dtypes
    from concourse.bass_utils import run_bass_kernel_spmd

    np_e3 = ml_dtypes.float8_e3m4
    np_bf = ml_dtypes.bfloat16

    h = np.asarray(hazard_pred, dtype=np.float32)
    t = np.asarray(times, dtype=np.float32)
    e = np.asarray(events, dtype=np.int32)
    assert h.shape == (N,)

    # ---- host bookkeeping: ordering + tie structure (integer only) ----
    order = np.argsort(t, kind="stable")
    t_s = t[order]
    h_s = h[order]
    e_s = e[order]
    first = np.searchsorted(t_s, t_s, side="left")     # group-start index
    n_at_start = np.bincount(first, weights=e_s.astype(np.float64),
                             minlength=N)              # events per group
    m = n_at_start[first]                              # broadcast to members
    w = (e_s * m).astype(np.float32)                   # e_i * n_g(i)
    cvec = np.zeros(N, dtype=np.float32)
    starts = first == np.arange(N)
    cvec[starts] = (n_at_start[starts] ** 2).astype(np.float32)
    n_events = int(e.sum())

    # time-DESCENDING, (core, block, partition, column) order, materialized
    # as the per-core SBUF image [P, C]
    hd = h_s[::-1].reshape(CORES, N // CORES)
    wd = w[::-1].reshape(CORES, N // CORES)
    cd = cvec[::-1].reshape(CORES, N // CORES)
    h8 = _image_layout(hd).astype(np_e3)
    w8 = _image_layout(wd).astype(np_e3)
    c16 = _image_layout(cd).astype(np_bf)

    tri = np.triu(np.ones((P, P), dtype=np.float32), 1)  # [k,i]=1 iff k<i

    nc1, nc2 = _get_programs()
    core_ids = list(range(CORES))

    in1 = [{"h": np.ascontiguousarray(h8[i]),
            "w": np.ascontiguousarray(w8[i])}
           for i in range(CORES)]
    r1 = run_bass_kernel_spmd(nc1, in1, core_ids=core_ids)
    part1 = np.stack([r1.results[i]["part1"] for i in range(CORES)])
    E8 = [r1.results[i]["e8"] for i in range(CORES)]

    S = part1[:, :, 0:NPAIR].sum(axis=(1, 2), dtype=np.float64)  # per-core
    idx = np.arange(P)
    T1 = part1[:, idx, NPAIR + idx].sum(dtype=np.float64)
    SSQ = part1[:, idx, NPAIR + P + idx].sum(dtype=np.float64)

    # descending-order prefix offsets across cores (8 scalar adds)
    offs = np.concatenate([[0.0], np.cumsum(S)[:-1]]).astype(np.float32)

    def smalls(off):
        sm = np.ones((P, 2 * P + 1), dtype=np.float32)
        sm[:, 0:P] = tri
        sm[:, 2 * P] = off
        return sm

    in2 = [{"e8": np.ascontiguousarray(E8[i]),
            "c16": np.ascontiguousarray(c16[i]),
            "smalls": smalls(offs[i])}
           for i in range(CORES)]
    r2 = run_bass_kernel_spmd(nc2, in2, core_ids=core_ids)
    part2 = np.stack([r2.results[i]["part2"] for i in range(CORES)])
    T2 = (part2[:, idx, idx].sum(dtype=np.float64)
          + part2[:, idx, P + idx].sum(dtype=np.float64))

    LAST.clear()
    LAST.update({"r1": r1, "r2": r2})

    total = T1 - T2
    loss = -total / n_events + 1e-4 * np.sqrt(SSQ)
    return np.float32(loss)
